# revision 1
# baseline (speedup 1.0000x reference)
"""DeepseekECMoE (expert-choice MoE) Trainium2 kernel, 8-way expert-parallel.

Layout per core c (SPMD, differences only via inputs):
  - routed expert c for all 8 batches: gate (f32r matmul) -> softmax over E
    (DVE tree) -> exact top-256 per (b, e=c) via max8/max_index/match_replace
    -> dispatch via one-hot matmul (bf16) -> expert MLP (bf16 matmuls, exact
    erf-gelu on ACT) -> unweighted token outputs + scores + indices out.
  - shared expert for batch b=c (bf16 matmuls).
Host combines: scatter-add weighted expert outputs, transpose, add shared.
"""
import numpy as np
import ml_dtypes

import concourse.bass as bass
import concourse.tile as tile
from concourse import bacc, mybir
from concourse.bass2jax import install_neuronx_cc_hook, _bass_exec_p, partition_id_tensor

B, S, H, E = 8, 1024, 1024, 8
I, ISH, CAP = 2048, 2048, 256
P = 128
HC, SC, NI, NISH = H // P, S // P, I // P, ISH // P
N_CORES = 8
dt = mybir.dt
BF16 = ml_dtypes.bfloat16

_CACHE: dict = {}


def _build_nc(act_name="Gelu"):
    nc = bacc.Bacc("TRN2", target_bir_lowering=False, debug=False,
                   num_devices=N_CORES)

    # ---- DRAM I/O ----
    hidT = nc.dram_tensor("hidT", [B, H, S], dt.float32r, kind="ExternalInput")
    hidb = nc.dram_tensor("hidb", [B, S, H], dt.bfloat16, kind="ExternalInput")
    gw = nc.dram_tensor("gw", [P, HC * E], dt.float32r, kind="ExternalInput")
    esel = nc.dram_tensor("esel", [E, 1], dt.float32r, kind="ExternalInput")
    ones8 = nc.dram_tensor("ones8", [E, 1], dt.float32r, kind="ExternalInput")
    bsel = nc.dram_tensor("bsel", [E, E * P], dt.float32r, kind="ExternalInput")
    gut = nc.dram_tensor("gut", [2, NI, P, HC * P], dt.bfloat16, kind="ExternalInput")
    dpTb = nc.dram_tensor("dpTb", [I, H], dt.bfloat16, kind="ExternalInput")
    sgut = nc.dram_tensor("sgut", [2, NISH, P, HC * P], dt.bfloat16, kind="ExternalInput")
    hshb = nc.dram_tensor("hshb", [H, S], dt.bfloat16, kind="ExternalInput")
    sdTb = nc.dram_tensor("sdTb", [ISH, H], dt.bfloat16, kind="ExternalInput")

    w_out = nc.dram_tensor("w_out", [B, CAP, H], dt.float32, kind="ExternalOutput")
    scoreso = nc.dram_tensor("scoreso", [B, CAP], dt.float32, kind="ExternalOutput")
    idxo = nc.dram_tensor("idxo", [B, CAP], dt.uint32, kind="ExternalOutput")
    sh_out = nc.dram_tensor("sh_out", [S, H], dt.float32, kind="ExternalOutput")

    AF = mybir.ActivationFunctionType
    ACT = getattr(AF, act_name)
    from contextlib import ExitStack
    with tile.TileContext(nc) as tc:
        with ExitStack() as ctx:
            pool = lambda name, bufs, **kw: ctx.enter_context(
                tc.tile_pool(name=name, bufs=bufs, **kw))
            pconst = pool("consts", 1)
            phtstr = pool("htstr", 3)
            pexp = pool("exp", 2)
            pwork = pool("work", 1)
            prden = pool("rden", 1)
            proute = pool("route", 1)
            phsh = pool("hsh", 8)
            psw = pool("sw", 4)
            pactsh = pool("actsh", 16)
            pdstr = pool("dstr", 17)
            pactT = pool("actT", 16)
            ptok = pool("tok", 9)
            pM = pool("Mpool", 8)
            phstr = pool("hstr", 9)
            pguw = pool("guw", 4)
            pgel = pool("gel", 2)
            pwo = pool("wo", 3)
            psmall = pool("small", 2)
            pgu = pool("pgu", 2, space="PSUM")
            pdown = pool("pdown", 2, space="PSUM")
            ptokp = pool("ptokp", 2, space="PSUM")
            # ---- constants ----
            t_gw = pconst.tile([P, HC * E], dt.float32r)
            nc.sync.dma_start(t_gw[:], gw[:])
            t_esel = pconst.tile([E, 1], dt.float32r)
            nc.sync.dma_start(t_esel[:], esel[:])
            t_ones8 = pconst.tile([E, 1], dt.float32r)
            nc.sync.dma_start(t_ones8[:], ones8[:])
            t_bsel = pconst.tile([E, E * P], dt.float32r)
            nc.sync.dma_start(t_bsel[:], bsel[:])
            t_iot = pconst.tile([P, SC], dt.int32)
            nc.gpsimd.iota(t_iot[:], pattern=[[P, SC]], base=0, channel_multiplier=1)
            t_iotf = pconst.tile([P, SC], dt.float32)
            nc.vector.tensor_copy(t_iotf[:], t_iot[:])

            # ---- gate + routing ----
            afftile = proute.tile([E, S], dt.float32)
            t_scores = proute.tile([E, CAP], dt.float32)
            t_idxu = proute.tile([E, CAP], dt.uint32)
            t_idxf = proute.tile([E, CAP], dt.float32)
            t_idxfr = proute.tile([E, CAP], dt.float32r)

            for b in range(B):
                exp_b = pexp.tile([E, S], dt.float32r)
                for sblk in range(2):
                    pl = ptokp.tile([E, 512], dt.float32, tag="ptk", name="pl")
                    for hc in range(HC):
                        ht = phtstr.tile([P, 512], dt.float32r)
                        nc.sync.dma_start(
                            ht[:], hidT[b, hc * P:(hc + 1) * P,
                                        sblk * 512:(sblk + 1) * 512])
                        nc.tensor.matmul(pl[:], t_gw[:, hc * E:(hc + 1) * E],
                                         ht[:], start=(hc == 0), stop=(hc == HC - 1))
                    nc.scalar.activation(exp_b[:, sblk * 512:(sblk + 1) * 512],
                                         pl[:], AF.Exp)
                rden = prden.tile([1, S], dt.float32)
                affrow = pwork.tile([1, S], dt.float32, tag="rt", name="affrow")
                for sblk in range(2):
                    sl = slice(sblk * 512, (sblk + 1) * 512)
                    pden = ptokp.tile([1, 512], dt.float32, tag="ptk", name="pden")
                    nc.tensor.matmul(pden[:], t_ones8[:], exp_b[:, sl],
                                     start=True, stop=True)
                    nc.vector.reciprocal(rden[:, sl], pden[:])
                    psel = ptokp.tile([1, 512], dt.float32, tag="ptk", name="psel")
                    nc.tensor.matmul(psel[:], t_esel[:], exp_b[:, sl],
                                     start=True, stop=True)
                    nc.vector.tensor_mul(affrow[:, sl], psel[:], rden[:, sl])
                nc.sync.dma_start(afftile[b:b + 1, :], affrow[:])

            for i in range(CAP // 8):
                sc8 = t_scores[:, i * 8:(i + 1) * 8]
                nc.vector.max(sc8, afftile[:])
                nc.vector.max_index(t_idxu[:, i * 8:(i + 1) * 8], sc8, afftile[:])
                nc.vector.match_replace(afftile[:], sc8, afftile[:], -1e30)
            nc.sync.dma_start(scoreso[:], t_scores[:])
            nc.sync.dma_start(idxo[:], t_idxu[:])
            nc.vector.tensor_copy(t_idxf[:], t_idxu[:])
            nc.vector.tensor_copy(t_idxfr[:], t_idxf[:])

            # ---- shared expert (batch c fed via hshb) ----
            hsh = []
            for hc in range(HC):
                t = phsh.tile([P, S], dt.bfloat16, tag="hsh", name="hsh")
                nc.sync.dma_start(t[:], hshb[hc * P:(hc + 1) * P, :])
                hsh.append(t)
            actsh = []
            for i in range(NISH):
                sg = psw.tile([P, HC * P], dt.bfloat16, bufs=2)
                nc.sync.dma_start(sg[:], sgut[0, i])
                su = psw.tile([P, HC * P], dt.bfloat16, bufs=2)
                nc.sync.dma_start(su[:], sgut[1, i])
                a = pactsh.tile([P, S], dt.bfloat16)
                for sblk in range(2):
                    pg = pgu.tile([P, 512], dt.float32, tag="pg", name="pg", bufs=2)
                    for hc in range(HC):
                        nc.tensor.matmul(pg[:], sg[:, hc * P:(hc + 1) * P],
                                         hsh[hc][:, sblk * 512:(sblk + 1) * 512],
                                         start=(hc == 0), stop=(hc == HC - 1))
                    pu = pgu.tile([P, 512], dt.float32, tag="pu", name="pu", bufs=2)
                    for hc in range(HC):
                        nc.tensor.matmul(pu[:], su[:, hc * P:(hc + 1) * P],
                                         hsh[hc][:, sblk * 512:(sblk + 1) * 512],
                                         start=(hc == 0), stop=(hc == HC - 1))
                    gel = pgel.tile([P, 512], dt.float32)
                    nc.scalar.activation(gel[:], pg[:], ACT)
                    nc.vector.tensor_mul(a[:, sblk * 512:(sblk + 1) * 512],
                                         gel[:], pu[:])
                actsh.append(a)
            sdt = []
            for ic in range(NISH):
                t = pdstr.tile([P, H], dt.bfloat16, tag="dstr", name="dstr")
                nc.sync.dma_start(t[:], sdTb[ic * P:(ic + 1) * P, :])
                sdt.append(t)
            for sblk in range(SC):
                for hh in range(2):
                    pd = pdown.tile([P, 512], dt.float32)
                    for ic in range(NISH):
                        nc.tensor.matmul(pd[:],
                                         actsh[ic][:, sblk * P:(sblk + 1) * P],
                                         sdt[ic][:, hh * 512:(hh + 1) * 512],
                                         start=(ic == 0), stop=(ic == NISH - 1))
                    sho = pwo.tile([P, 512], dt.float32, tag="wo", name="wo")
                    nc.scalar.copy(sho[:], pd[:])
                    nc.sync.dma_start(
                        sh_out[sblk * P:(sblk + 1) * P, hh * 512:(hh + 1) * 512],
                        sho[:])

            # ---- routed expert, batch pairs ----
            for pair in range(B // 2):
                b0 = 2 * pair
                tokT = []
                for hc in range(HC):
                    tokT.append(ptok.tile([P, 2 * CAP], dt.bfloat16, tag="tokT", name="tokT"))
                for bi in range(2):
                    b = b0 + bi
                    pib = ptokp.tile([P, CAP], dt.float32, tag="ptk", name="pib")
                    nc.tensor.matmul(pib[:], t_bsel[:, b * P:(b + 1) * P],
                                     t_idxfr[:], start=True, stop=True)
                    idxB = psmall.tile([P, CAP], dt.float32)
                    nc.vector.tensor_copy(idxB[:], pib[:])
                    Ms = []
                    for sc in range(SC):
                        m = pM.tile([P, CAP], dt.bfloat16, tag="M", name="M")
                        nc.vector.tensor_scalar(m[:], idxB[:], t_iotf[:, sc:sc + 1],
                                                None, mybir.AluOpType.is_equal)
                        Ms.append(m)
                    hh_tiles = []
                    for sc in range(SC):
                        t = phstr.tile([P, H], dt.bfloat16, tag="hstr", name="hstr")
                        nc.sync.dma_start(t[:], hidb[b, sc * P:(sc + 1) * P, :])
                        hh_tiles.append(t)
                    for hblk in range(HC):
                        pt = ptokp.tile([P, CAP], dt.float32, tag="ptk", name="pt")
                        for sc in range(SC):
                            nc.tensor.matmul(pt[:],
                                             hh_tiles[sc][:, hblk * P:(hblk + 1) * P],
                                             Ms[sc][:],
                                             start=(sc == 0), stop=(sc == SC - 1))
                        nc.vector.tensor_copy(
                            tokT[hblk][:, bi * CAP:(bi + 1) * CAP], pt[:])

                actT = []
                for i in range(NI):
                    sg = pguw.tile([P, HC * P], dt.bfloat16, bufs=2)
                    nc.sync.dma_start(sg[:], gut[0, i])
                    su = pguw.tile([P, HC * P], dt.bfloat16, bufs=2)
                    nc.sync.dma_start(su[:], gut[1, i])
                    pg = pgu.tile([P, 2 * CAP], dt.float32, tag="pg", name="pg", bufs=2)
                    for hc in range(HC):
                        nc.tensor.matmul(pg[:], sg[:, hc * P:(hc + 1) * P],
                                         tokT[hc][:],
                                         start=(hc == 0), stop=(hc == HC - 1))
                    pu = pgu.tile([P, 2 * CAP], dt.float32, tag="pu", name="pu", bufs=2)
                    for hc in range(HC):
                        nc.tensor.matmul(pu[:], su[:, hc * P:(hc + 1) * P],
                                         tokT[hc][:],
                                         start=(hc == 0), stop=(hc == HC - 1))
                    gel = pgel.tile([P, 2 * CAP], dt.float32)
                    nc.scalar.activation(gel[:], pg[:], ACT)
                    a = pactT.tile([P, 2 * CAP], dt.bfloat16)
                    nc.vector.tensor_mul(a[:], gel[:], pu[:])
                    actT.append(a)

                dpt = []
                for ic in range(NI):
                    t = pdstr.tile([P, H], dt.bfloat16, tag="dstr", name="dstr")
                    nc.sync.dma_start(t[:], dpTb[ic * P:(ic + 1) * P, :])
                    dpt.append(t)
                for tb in range(4):
                    b = b0 + tb // 2
                    rblk = tb % 2
                    for hh in range(2):
                        pd = pdown.tile([P, 512], dt.float32)
                        for ic in range(NI):
                            nc.tensor.matmul(pd[:],
                                             actT[ic][:, tb * P:(tb + 1) * P],
                                             dpt[ic][:, hh * 512:(hh + 1) * 512],
                                             start=(ic == 0), stop=(ic == NI - 1))
                        wo = pwo.tile([P, 512], dt.float32, tag="wo", name="wo")
                        nc.scalar.copy(wo[:], pd[:])
                        nc.sync.dma_start(
                            w_out[b, rblk * P:(rblk + 1) * P,
                                  hh * 512:(hh + 1) * 512], wo[:])

    nc.compile()
    return nc


class _Exec:
    """Cached multi-core PJRT executor (mirrors bass2jax.run_bass_via_pjrt)."""

    def __init__(self, nc):
        import jax
        from jax.sharding import Mesh, PartitionSpec
        from jax.experimental.shard_map import shard_map

        install_neuronx_cc_hook()
        self.nc = nc
        in_names, out_names, out_avals = [], [], []
        partition_name = (nc.partition_id_tensor.name
                          if nc.partition_id_tensor else None)
        for alloc in nc.m.functions[0].allocations:
            if not isinstance(alloc, mybir.MemoryLocationSet):
                continue
            name = alloc.memorylocations[0].name
            if alloc.kind == "ExternalInput":
                if name != partition_name:
                    in_names.append(name)
            elif alloc.kind == "ExternalOutput":
                out_names.append(name)
                out_avals.append(jax.core.ShapedArray(
                    tuple(alloc.tensor_shape), mybir.dt.np(alloc.dtype)))
        self.in_names, self.out_names, self.out_avals = in_names, out_names, out_avals
        self.partition_name = partition_name
        n_params = len(in_names)
        n_outs = len(out_names)
        all_in_names = list(in_names) + list(out_names)
        if partition_name is not None:
            all_in_names.append(partition_name)

        def _body(*args):
            operands = list(args)
            if partition_name is not None:
                operands.append(partition_id_tensor())
            outs = _bass_exec_p.bind(
                *operands,
                out_avals=tuple(out_avals),
                in_names=tuple(all_in_names),
                out_names=tuple(out_names),
                lowering_input_output_aliases=(),
                sim_require_finite=True,
                sim_require_nnan=True,
                nc=nc,
            )
            return tuple(outs)

        devices = jax.devices()[:N_CORES]
        mesh = Mesh(np.asarray(devices), ("core",))
        in_specs = (PartitionSpec("core"),) * (n_params + n_outs)
        out_specs = (PartitionSpec("core"),) * n_outs
        self.sharded = jax.jit(
            shard_map(_body, mesh=mesh, in_specs=in_specs, out_specs=out_specs,
                      check_rep=False),
            donate_argnums=tuple(range(n_params, n_params + n_outs)),
            keep_unused=True,
        )

    def concat_inputs(self, in_maps):
        return [
            np.concatenate([np.asarray(in_maps[c][name]) for c in range(N_CORES)],
                           axis=0)
            for name in self.in_names
        ]

    def zero_outs(self):
        return [np.zeros((N_CORES * a.shape[0], *a.shape[1:]), a.dtype)
                for a in self.out_avals]

    def run_raw(self, concat_in):
        return self.sharded(*concat_in, *self.zero_outs())

    def run(self, in_maps):
        out_arrs = self.run_raw(self.concat_inputs(in_maps))
        return [
            {name: np.asarray(out_arrs[i]).reshape(N_CORES, *self.out_avals[i].shape)[c]
             for i, name in enumerate(self.out_names)}
            for c in range(N_CORES)
        ]


def _get_exec():
    if "exec" not in _CACHE:
        _CACHE["exec"] = _Exec(_build_nc())
    return _CACHE["exec"]


def _prep_in_maps(hidden_states, gate_w, gate_proj, up_proj, down_proj,
                  s_gate, s_up, s_down):
    f32 = np.float32
    hid = np.ascontiguousarray(hidden_states, dtype=f32)
    hidT = np.ascontiguousarray(hid.transpose(0, 2, 1))
    hidb = hid.astype(BF16)
    gw = np.ascontiguousarray(
        np.asarray(gate_w, f32).reshape(HC, P, E).transpose(1, 0, 2).reshape(P, HC * E))
    ones8 = np.ones((E, 1), f32)
    bselm = np.zeros((E, E * P), f32)
    for b in range(E):
        bselm[b, b * P:(b + 1) * P] = 1.0

    def tile_gu(gT):  # gT [H, X] -> [X//P, P, HC*P]
        X = gT.shape[1]
        return np.ascontiguousarray(
            gT.reshape(HC, P, X // P, P).transpose(2, 1, 0, 3).reshape(X // P, P, HC * P))

    sgT = np.asarray(s_gate, f32).T  # [H, ISH]
    suT = np.asarray(s_up, f32).T
    sgut = np.stack([tile_gu(sgT), tile_gu(suT)]).astype(BF16)
    sdTb = np.ascontiguousarray(np.asarray(s_down, f32).T).astype(BF16)  # [ISH, H]

    gp = np.asarray(gate_proj, f32)
    up = np.asarray(up_proj, f32)
    dn = np.asarray(down_proj, f32)

    in_maps = []
    for c in range(N_CORES):
        gpT = gp[c].T  # [H, I]
        upT = up[c].T
        gut = np.stack([tile_gu(gpT), tile_gu(upT)]).astype(BF16)
        dpTb = np.ascontiguousarray(dn[c].T).astype(BF16)  # [I, H]
        es = np.zeros((E, 1), f32)
        es[c, 0] = 1.0
        in_maps.append({
            "hidT": hidT, "hidb": hidb, "gw": gw, "esel": es,
            "ones8": ones8, "bsel": bselm,
            "gut": gut, "dpTb": dpTb, "sgut": sgut,
            "hshb": hidT[c].astype(BF16), "sdTb": sdTb,
        })
    return in_maps


def _combine(results):
    f32 = np.float32
    comb = np.zeros((B, S, H), f32)
    b_ix = np.arange(B)[:, None]
    for c in range(N_CORES):
        r = results[c]
        w = r["w_out"] * r["scoreso"][:, :, None]
        comb[b_ix, r["idxo"].astype(np.int64)] += w
    shared = np.stack([results[c]["sh_out"] for c in range(N_CORES)])
    return comb.transpose(0, 2, 1) + shared


def kernel(**inputs):
    ex = _get_exec()
    in_maps = _prep_in_maps(**inputs)
    results = ex.run(in_maps)
    return _combine(results).astype(np.float32)



# revision 9
# speedup vs baseline: 913.8144x; 913.8144x over previous
"""DeepseekECMoE (expert-choice MoE) Trainium2 kernel, 8-way expert-parallel.

Layout per core c (SPMD, differences only via inputs):
  - gate (f32r matmul, all batches) -> softmax row for expert c -> exact
    top-256 per batch via max8/max_index/match_replace -> token gather via
    indirect DMA -> expert MLP (bf16 matmuls, erf-gelu on ACT), expert
    weights loaded once -> score-weighted bf16 token outputs + indices out.
  - shared expert for batch b=c (bf16 matmuls), bf16 output.
Emission order puts the serial top-k DVE chain after the shared-expert
multiplies so it overlaps the shared-expert down projection on PE.
Host combines: scatter-add weighted expert outputs, transpose, add shared.
"""
import numpy as np
import ml_dtypes

import concourse.bass as bass
import concourse.tile as tile
from concourse import bacc, mybir
from concourse.bass2jax import install_neuronx_cc_hook, _bass_exec_p, partition_id_tensor
from concourse.masks import make_identity

B, S, H, E = 8, 1024, 1024, 8
I, ISH, CAP = 2048, 2048, 256
P = 128
HC, SC, NI, NISH = H // P, S // P, I // P, ISH // P
N_CORES = 8
dt = mybir.dt
BF16 = ml_dtypes.bfloat16

_CACHE: dict = {}


def _build_nc(act_name="Gelu"):
    nc = bacc.Bacc("TRN2", target_bir_lowering=False, debug=False,
                   num_devices=N_CORES)

    # ---- DRAM I/O ----
    hidT = nc.dram_tensor("hidT", [B, H, S], dt.float32r, kind="ExternalInput")
    hidf = nc.dram_tensor("hidf", [B * S, H], dt.bfloat16, kind="ExternalInput")
    gw = nc.dram_tensor("gw", [P, HC * E], dt.float32r, kind="ExternalInput")
    esel = nc.dram_tensor("esel", [E, 1], dt.float32r, kind="ExternalInput")
    ones8 = nc.dram_tensor("ones8", [E, 1], dt.float32r, kind="ExternalInput")
    gut = nc.dram_tensor("gut", [2, NI, P, HC * P], dt.bfloat16, kind="ExternalInput")
    dpTb = nc.dram_tensor("dpTb", [I, H], dt.bfloat16, kind="ExternalInput")
    sgut = nc.dram_tensor("sgut", [2, NISH, P, HC * P], dt.bfloat16, kind="ExternalInput")
    sdTb = nc.dram_tensor("sdTb", [ISH, H], dt.bfloat16, kind="ExternalInput")
    hshb = nc.dram_tensor("hshb", [H, S], dt.bfloat16, kind="ExternalInput")

    w_out = nc.dram_tensor("w_out", [B, CAP, H], dt.bfloat16, kind="ExternalOutput")
    idxo = nc.dram_tensor("idxo", [B, CAP], dt.uint32, kind="ExternalOutput")
    sh_out = nc.dram_tensor("sh_out", [S, H], dt.bfloat16, kind="ExternalOutput")

    AF = mybir.ActivationFunctionType
    ACT = getattr(AF, act_name)
    from contextlib import ExitStack
    with tile.TileContext(nc) as tc:
        with ExitStack() as ctx:
            pool = lambda name, bufs, **kw: ctx.enter_context(
                tc.tile_pool(name=name, bufs=bufs, **kw))
            pconst = pool("consts", 1)
            phts = pool("hts", 3)
            pexp = pool("exp", 2)
            pwork = pool("work", 1)
            prden = pool("rden", 1)
            proute = pool("route", 1)
            phsh = pool("hsh", 8)
            pguw = pool("guw", 4)
            pacts = pool("acts", 16)
            pdownw = pool("downw", 16)
            ptokT = pool("tokT", 16)
            pgath = pool("gath", 3)
            pgel = pool("gel", 2)
            pwo = pool("wo", 4)
            psmallT = pool("smallT", 2)
            # PSUM: 8 banks total
            pps = pool("pps", 2, space="PSUM")     # gate + down proj
            pgu = pool("pgu", 2, space="PSUM")     # gate/up matmuls (pg+pu tags)
            ptr = pool("ptr", 2, space="PSUM")     # transposes

            # ---- constants ----
            t_gw = pconst.tile([P, HC * E], dt.float32r)
            nc.sync.dma_start(t_gw[:], gw[:])
            t_esel = pconst.tile([E, 1], dt.float32r)
            nc.sync.dma_start(t_esel[:], esel[:])
            t_ones8 = pconst.tile([E, 1], dt.float32r)
            nc.sync.dma_start(t_ones8[:], ones8[:])
            t_idb = pconst.tile([P, P], dt.bfloat16)
            make_identity(nc, t_idb[:])
            t_idf8 = pconst.tile([E, E], dt.float32)
            make_identity(nc, t_idf8[:])
            t_iotB = pconst.tile([E, 1], dt.int32)
            nc.gpsimd.iota(t_iotB[:], pattern=[[0, 1]], base=0,
                           channel_multiplier=S)
            t_iotBf = pconst.tile([E, 1], dt.float32)
            nc.vector.tensor_copy(t_iotBf[:], t_iotB[:])

            # ---- gate: softmax row for expert c, all batches ----
            afftile = proute.tile([E, S], dt.float32)
            hsh = []
            for hc in range(HC):
                t = phsh.tile([P, S], dt.bfloat16, tag="hsh", name="hsh")
                nc.sync.dma_start(t[:], hshb[hc * P:(hc + 1) * P, :])
                hsh.append(t)
            for b in range(B):
                exp_b = pexp.tile([E, S], dt.float32r)
                for sblk in range(2):
                    pl = pps.tile([P, 512], dt.float32, tag="pps", name="pl")
                    for hc in range(HC):
                        ht = phts.tile([P, 512], dt.float32r)
                        nc.sync.dma_start(
                            ht[:], hidT[b, hc * P:(hc + 1) * P,
                                        sblk * 512:(sblk + 1) * 512])
                        nc.tensor.matmul(pl[:E], t_gw[:, hc * E:(hc + 1) * E],
                                         ht[:], start=(hc == 0), stop=(hc == HC - 1))
                    nc.scalar.activation(exp_b[:, sblk * 512:(sblk + 1) * 512],
                                         pl[:E], AF.Exp)
                rden = prden.tile([1, S], dt.float32)
                affrow = pwork.tile([1, S], dt.float32, tag="rt", name="affrow")
                for sblk in range(2):
                    sl = slice(sblk * 512, (sblk + 1) * 512)
                    pden = pps.tile([P, 512], dt.float32, tag="pps", name="pden")
                    nc.tensor.matmul(pden[:1], t_ones8[:], exp_b[:, sl],
                                     start=True, stop=True)
                    nc.vector.reciprocal(rden[:, sl], pden[:1])
                    psel = pps.tile([P, 512], dt.float32, tag="pps", name="psel")
                    nc.tensor.matmul(psel[:1], t_esel[:], exp_b[:, sl],
                                     start=True, stop=True)
                    nc.vector.tensor_mul(affrow[:, sl], psel[:1], rden[:, sl])
                nc.sync.dma_start(afftile[b:b + 1, :], affrow[:])

            # ---- shared expert gate/up (batch c hidden fed via hshdma) ----
            actsh = []
            for i in range(NISH):
                sg = pguw.tile([P, HC * P], dt.bfloat16, name="sg")
                nc.sync.dma_start(sg[:], sgut[0, i])
                su = pguw.tile([P, HC * P], dt.bfloat16, name="su")
                nc.sync.dma_start(su[:], sgut[1, i])
                a = pacts.tile([P, 2 * S], dt.bfloat16, tag="acts", name="acts")
                for sblk in range(2):
                    pg = pgu.tile([P, 512], dt.float32, tag="pg", name="pg", bufs=2)
                    for hc in range(HC):
                        nc.tensor.matmul(pg[:], sg[:, hc * P:(hc + 1) * P],
                                         hsh[hc][:, sblk * 512:(sblk + 1) * 512],
                                         start=(hc == 0), stop=(hc == HC - 1))
                    pu = pgu.tile([P, 512], dt.float32, tag="pu", name="pu", bufs=2)
                    for hc in range(HC):
                        nc.tensor.matmul(pu[:], su[:, hc * P:(hc + 1) * P],
                                         hsh[hc][:, sblk * 512:(sblk + 1) * 512],
                                         start=(hc == 0), stop=(hc == HC - 1))
                    gel = pgel.tile([P, 512], dt.float32)
                    nc.scalar.activation(gel[:], pg[:], ACT)
                    nc.vector.tensor_mul(a[:, sblk * 512:(sblk + 1) * 512],
                                         gel[:], pu[:])
                actsh.append(a)
            sdt = []
            for ic in range(NISH):
                t = pdownw.tile([P, H], dt.bfloat16, tag="dw", name="dw")
                nc.sync.dma_start(t[:], sdTb[ic * P:(ic + 1) * P, :])
                sdt.append(t)
            for sblk in range(SC):
                for hh in range(2):
                    pd = pps.tile([P, 512], dt.float32, tag="pps", name="pd")
                    for ic in range(NISH):
                        nc.tensor.matmul(pd[:],
                                         actsh[ic][:, sblk * P:(sblk + 1) * P],
                                         sdt[ic][:, hh * 512:(hh + 1) * 512],
                                         start=(ic == 0), stop=(ic == NISH - 1))
                    sho = pwo.tile([P, 512], dt.bfloat16, tag="wo", name="wo")
                    nc.scalar.copy(sho[:], pd[:])
                    nc.sync.dma_start(
                        sh_out[sblk * P:(sblk + 1) * P, hh * 512:(hh + 1) * 512],
                        sho[:])

            # ---- top-k (serial DVE chain; overlaps shared down on PE) ----
            t_scores = proute.tile([E, CAP], dt.float32)
            t_idxu = proute.tile([E, CAP], dt.uint32)
            t_idxf = proute.tile([E, CAP], dt.float32)
            t_idxg = proute.tile([E, CAP], dt.float32)
            for i in range(CAP // 8):
                sc8 = t_scores[:, i * 8:(i + 1) * 8]
                nc.vector.max(sc8, afftile[:])
                nc.vector.max_index(t_idxu[:, i * 8:(i + 1) * 8], sc8, afftile[:])
                nc.vector.match_replace(afftile[:], sc8, afftile[:], -1e30)
            nc.sync.dma_start(idxo[:], t_idxu[:])
            nc.vector.tensor_copy(t_idxf[:], t_idxu[:])
            nc.vector.tensor_scalar(t_idxg[:], t_idxf[:], t_iotBf[:, :1],
                                    None, mybir.AluOpType.add)

            # transpose scores + global indices to per-token columns
            scT = [psmallT.tile([P, E], dt.float32, tag="scT", name="scT")
                   for _ in range(2)]
            idxT = [psmallT.tile([P, E], dt.uint32, tag="idxT", name="idxT")
                    for _ in range(2)]
            for half in range(2):
                ptp = ptr.tile([P, P], dt.float32, tag="ptr", name="ptr")
                nc.tensor.transpose(ptp[:, :E],
                                    t_scores[:, half * P:(half + 1) * P],
                                    t_idf8[:])
                nc.vector.tensor_copy(scT[half][:], ptp[:, :E])
                ptq = ptr.tile([P, P], dt.float32, tag="ptr", name="ptr")
                nc.tensor.transpose(ptq[:, :E],
                                    t_idxg[:, half * P:(half + 1) * P],
                                    t_idf8[:])
                nc.vector.tensor_copy(idxT[half][:], ptq[:, :E])

            # ---- dispatch: indirect gather + PE transpose into tokT ----
            # tokT tile [hc][half8] holds h-block hc for 8 column-tiles of
            # 128 tokens each: j = 2*b + half -> tokens half*128.. of batch b
            tokT = [[ptokT.tile([P, 8 * P], dt.bfloat16, tag="tokT", name="tokT")
                     for _ in range(2)] for _ in range(HC)]
            for j in range(16):
                b, half = j // 2, j % 2
                g = pgath.tile([P, H], dt.bfloat16, tag="g", name="g")
                nc.gpsimd.indirect_dma_start(
                    out=g[:], out_offset=None, in_=hidf[:],
                    in_offset=bass.IndirectOffsetOnAxis(
                        ap=idxT[half][:, b:b + 1], axis=0))
                for hc in range(HC):
                    ptp = ptr.tile([P, P], dt.bfloat16, tag="ptr", name="ptr")
                    nc.tensor.transpose(ptp[:], g[:, hc * P:(hc + 1) * P],
                                        t_idb[:])
                    nc.scalar.copy(tokT[hc][half][:, b * P:(b + 1) * P], ptp[:])

            # ---- routed expert MLP, weights loaded once ----
            actT = []
            for i in range(NI):
                sg = pguw.tile([P, HC * P], dt.bfloat16, name="sg")
                nc.sync.dma_start(sg[:], gut[0, i])
                su = pguw.tile([P, HC * P], dt.bfloat16, name="su")
                nc.sync.dma_start(su[:], gut[1, i])
                a = pacts.tile([P, 2 * S], dt.bfloat16, tag="acts", name="acts")
                for ch in range(4):  # 512-token chunks
                    half, col = ch // 2, (ch % 2) * 512
                    pg = pgu.tile([P, 512], dt.float32, tag="pg", name="pg", bufs=2)
                    for hc in range(HC):
                        nc.tensor.matmul(pg[:], sg[:, hc * P:(hc + 1) * P],
                                         tokT[hc][half][:, col:col + 512],
                                         start=(hc == 0), stop=(hc == HC - 1))
                    pu = pgu.tile([P, 512], dt.float32, tag="pu", name="pu", bufs=2)
                    for hc in range(HC):
                        nc.tensor.matmul(pu[:], su[:, hc * P:(hc + 1) * P],
                                         tokT[hc][half][:, col:col + 512],
                                         start=(hc == 0), stop=(hc == HC - 1))
                    gel = pgel.tile([P, 512], dt.float32)
                    nc.scalar.activation(gel[:], pg[:], ACT)
                    nc.vector.tensor_mul(a[:, ch * 512:(ch + 1) * 512],
                                         gel[:], pu[:])
                actT.append(a)

            dpt = []
            for ic in range(NI):
                t = pdownw.tile([P, H], dt.bfloat16, tag="dw", name="dw")
                nc.sync.dma_start(t[:], dpTb[ic * P:(ic + 1) * P, :])
                dpt.append(t)
            for tb in range(16):  # token tiles; tb = 2*b + half
                b, half = tb // 2, tb % 2
                acol = half * 8 * P + b * P  # actT layout is half-major
                for hh in range(2):
                    pd = pps.tile([P, 512], dt.float32, tag="pps", name="pd")
                    for ic in range(NI):
                        nc.tensor.matmul(pd[:],
                                         actT[ic][:, acol:acol + P],
                                         dpt[ic][:, hh * 512:(hh + 1) * 512],
                                         start=(ic == 0), stop=(ic == NI - 1))
                    wo = pwo.tile([P, 512], dt.bfloat16, tag="wo", name="wo")
                    nc.vector.tensor_scalar(wo[:], pd[:], scT[half][:, b:b + 1],
                                            None, mybir.AluOpType.mult)
                    nc.sync.dma_start(
                        w_out[b, half * P:(half + 1) * P,
                              hh * 512:(hh + 1) * 512], wo[:])

    nc.compile()
    return nc


class _Exec:
    """Cached multi-core PJRT executor (mirrors bass2jax.run_bass_via_pjrt)."""

    def __init__(self, nc):
        import jax
        from jax.sharding import Mesh, PartitionSpec
        from jax.experimental.shard_map import shard_map

        install_neuronx_cc_hook()
        self.nc = nc
        in_names, out_names, out_avals = [], [], []
        partition_name = (nc.partition_id_tensor.name
                          if nc.partition_id_tensor else None)
        for alloc in nc.m.functions[0].allocations:
            if not isinstance(alloc, mybir.MemoryLocationSet):
                continue
            name = alloc.memorylocations[0].name
            if alloc.kind == "ExternalInput":
                if name != partition_name:
                    in_names.append(name)
            elif alloc.kind == "ExternalOutput":
                out_names.append(name)
                out_avals.append(jax.core.ShapedArray(
                    tuple(alloc.tensor_shape), mybir.dt.np(alloc.dtype)))
        self.in_names, self.out_names, self.out_avals = in_names, out_names, out_avals
        self.partition_name = partition_name
        n_params = len(in_names)
        n_outs = len(out_names)
        all_in_names = list(in_names) + list(out_names)
        if partition_name is not None:
            all_in_names.append(partition_name)

        def _body(*args):
            operands = list(args)
            if partition_name is not None:
                operands.append(partition_id_tensor())
            outs = _bass_exec_p.bind(
                *operands,
                out_avals=tuple(out_avals),
                in_names=tuple(all_in_names),
                out_names=tuple(out_names),
                lowering_input_output_aliases=(),
                sim_require_finite=True,
                sim_require_nnan=True,
                nc=nc,
            )
            return tuple(outs)

        devices = jax.devices()[:N_CORES]
        mesh = Mesh(np.asarray(devices), ("core",))
        in_specs = (PartitionSpec("core"),) * (n_params + n_outs)
        out_specs = (PartitionSpec("core"),) * n_outs
        self.sharded = jax.jit(
            shard_map(_body, mesh=mesh, in_specs=in_specs, out_specs=out_specs,
                      check_rep=False),
            donate_argnums=tuple(range(n_params, n_params + n_outs)),
            keep_unused=True,
        )

    def concat_inputs(self, in_maps):
        return [
            np.concatenate([np.asarray(in_maps[c][name]) for c in range(N_CORES)],
                           axis=0)
            for name in self.in_names
        ]

    def zero_outs(self):
        return [np.zeros((N_CORES * a.shape[0], *a.shape[1:]), a.dtype)
                for a in self.out_avals]

    def run_raw(self, concat_in):
        return self.sharded(*concat_in, *self.zero_outs())

    def run(self, in_maps):
        out_arrs = self.run_raw(self.concat_inputs(in_maps))
        return [
            {name: np.asarray(out_arrs[i]).reshape(N_CORES, *self.out_avals[i].shape)[c]
             for i, name in enumerate(self.out_names)}
            for c in range(N_CORES)
        ]


def _get_exec():
    if "exec" not in _CACHE:
        _CACHE["exec"] = _Exec(_build_nc())
    return _CACHE["exec"]


def _prep_in_maps(hidden_states, gate_w, gate_proj, up_proj, down_proj,
                  s_gate, s_up, s_down):
    f32 = np.float32
    hid = np.ascontiguousarray(hidden_states, dtype=f32)
    hidT = np.ascontiguousarray(hid.transpose(0, 2, 1))
    hidf = np.ascontiguousarray(hid.reshape(B * S, H)).astype(BF16)
    gw = np.ascontiguousarray(
        np.asarray(gate_w, f32).reshape(HC, P, E).transpose(1, 0, 2).reshape(P, HC * E))
    ones8 = np.ones((E, 1), f32)

    def tile_gu(gT):  # gT [H, X] -> [X//P, P, HC*P]
        X = gT.shape[1]
        return np.ascontiguousarray(
            gT.reshape(HC, P, X // P, P).transpose(2, 1, 0, 3).reshape(X // P, P, HC * P))

    sgT = np.asarray(s_gate, f32).T  # [H, ISH]
    suT = np.asarray(s_up, f32).T
    sgut = np.stack([tile_gu(sgT), tile_gu(suT)]).astype(BF16)
    sdTb = np.ascontiguousarray(np.asarray(s_down, f32).T).astype(BF16)  # [ISH, H]

    gp = np.asarray(gate_proj, f32)
    up = np.asarray(up_proj, f32)
    dn = np.asarray(down_proj, f32)

    in_maps = []
    for c in range(N_CORES):
        gpT = gp[c].T  # [H, I]
        upT = up[c].T
        gutc = np.stack([tile_gu(gpT), tile_gu(upT)]).astype(BF16)
        dpTb = np.ascontiguousarray(dn[c].T).astype(BF16)  # [I, H]
        es = np.zeros((E, 1), f32)
        es[c, 0] = 1.0
        in_maps.append({
            "hidT": hidT, "hidf": hidf, "gw": gw, "esel": es,
            "ones8": ones8,
            "gut": gutc, "dpTb": dpTb, "sgut": sgut, "sdTb": sdTb,
            "hshb": hidT[c].astype(BF16),
        })
    return in_maps


def _combine(results):
    f32 = np.float32
    comb = np.zeros((B, S, H), f32)
    b_ix = np.arange(B)[:, None]
    for c in range(N_CORES):
        r = results[c]
        idx = r["idxo"].astype(np.int64)
        comb[b_ix, idx] += r["w_out"].astype(f32)
    shared = np.stack([results[c]["sh_out"].astype(f32) for c in range(N_CORES)])
    return comb.transpose(0, 2, 1) + shared


def kernel(**inputs):
    ex = _get_exec()
    in_maps = _prep_in_maps(**inputs)
    results = ex.run(in_maps)
    return _combine(results).astype(np.float32)
